# revision 7
# baseline (speedup 1.0000x reference)
"""GroupInfoNCE loss kernel for 8 Trainium2 NeuronCores.

Strategy (row-sharded, f2 replicated + column-rotated, no collectives):
  - Host normalizes f1/f2 in f32, pre-scales x16, quantizes to fp8e4m3,
    transposes to [D, rows]. Core k receives its f1 row-shard [256, 1024]
    plus the FULL f2T [256, 8192] with columns rotated left by 1024*k, so
    every core runs an identical program whose jb=0 column block is its
    own diagonal (positive) block. No AllGather, no collective rendezvous.
  - The 8192x8192 logits matrix never touches HBM: each [128,1024] GEMM
    tile (fp8 DoubleRow) is consumed in PSUM by ScalarE exp -> bf16.
  - Row stats: VectorE 3D-AP reduce -> 16-wide block sums rowblk; row
    positives come from rowblk's jb=0 columns via a mask-multiply.
  - Column stats: TensorE ones-matmul -> per-core column block sums craw
    [64, 1024] per jb (bf16, copied PSUM->SBUF on the idle Pool engine);
    64->1 partition reduction via tiny bf16 ones-matmuls on PE. Column
    positives = craw0 masked to its block-diagonal, same ones-matmul.
  - All Ln ops are pinned after the Exp stream via a late-data bias
    operand so ScalarE swaps activation tables exactly once.
  - Host does the tiny O(GN) combine (un-rotating column stats).
"""

import os
import numpy as np

GN, D = 8192, 256
NGRP = 16               # group length N
EPS = 0.1               # label smoothing
G = GN // NGRP          # 512 groups
NCORES = 8
RPC = GN // NCORES      # 1024 rows per core
NSTRIP = RPC // 128     # 8 strips of 128 rows
NJB = GN // 1024        # 8 j-blocks of 1024 columns

_cache = {}
last_results = None


def _build_program(scale: float):
    PRE = 16.0  # host fp8 pre-scale; folded out of the exp activation scale
    from contextlib import ExitStack
    import concourse.bass as bass  # noqa: F401
    import concourse.mybir as mybir
    import concourse.tile as tile
    from concourse import bacc

    f32 = mybir.dt.float32
    bf16 = mybir.dt.bfloat16
    f8 = mybir.dt.float8e4
    AF = mybir.ActivationFunctionType
    AX = mybir.AxisListType
    ALU = mybir.AluOpType

    nc = bacc.Bacc(
        "TRN2",
        target_bir_lowering=False,
        debug=False,
        enable_asserts=False,
        num_devices=NCORES,
    )

    # packed input: cols [0, RPC) = f1T shard, cols [RPC, RPC+GN) = rotated f2T
    fin_d = nc.dram_tensor("fin", [D, RPC + GN], f8, kind="ExternalInput").ap()

    # packed outputs: o_rows = [asum | slog | pos] column-wise
    o_rows_d = nc.dram_tensor(
        "o_rows", [128, 3 * NSTRIP], f32, kind="ExternalOutput"
    ).ap()
    o_pos2_d = nc.dram_tensor("o_pos2", [1, RPC], f32, kind="ExternalOutput").ap()
    o_cab_d = nc.dram_tensor("o_cab", [2, GN], f32, kind="ExternalOutput").ap()

    with tile.TileContext(nc) as tc, ExitStack() as ctx:
        singles = ctx.enter_context(tc.tile_pool(name="singles", bufs=1))
        expp = ctx.enter_context(tc.tile_pool(name="expp", bufs=8))
        crawp = ctx.enter_context(tc.tile_pool(name="crawp", bufs=2))
        scratch = ctx.enter_context(tc.tile_pool(name="scratch", bufs=2))

        # -------- constants built on device (no input bytes) --------
        # mask128[p, g] = 1 iff g == p//16, via two affine selects on the
        # iota value v(p, g) = p - 16g (keep 0 <= v <= 15)
        ones8 = singles.tile([128, 8], f32, name="ones8")
        nc.vector.memset(ones8, 1.0)
        mtmp = singles.tile([128, 8], f32, name="mtmp")
        nc.gpsimd.affine_select(
            mtmp, ones8, pattern=[[-16, 8]], compare_op=ALU.is_ge,
            fill=0.0, base=0, channel_multiplier=1,
        )
        mask128_sb = singles.tile([128, 8], f32, name="mask128_sb")
        nc.gpsimd.affine_select(
            mask128_sb, mtmp, pattern=[[16, 8]], compare_op=ALU.is_ge,
            fill=0.0, base=15, channel_multiplier=-1,
        )
        # ones64[p, t, c] = 1 iff c == 8t + p//16  (colsum weights per strip)
        ones64_sb = singles.tile([128, NSTRIP, 64], bf16, name="ones64_sb")
        nc.vector.memset(ones64_sb, 0.0)
        for t in range(NSTRIP):
            nc.vector.tensor_copy(
                ones64_sb[:, t, 8 * t : 8 * t + 8], mask128_sb
            )
        # mask64[g, c] = 1 iff c//16 == g (extracts craw0's positive entries)
        ones1k = singles.tile([64, 1024], bf16, name="ones1k")
        nc.vector.memset(ones1k, 1.0)
        m64tmp = singles.tile([64, 1024], bf16, name="m64tmp")
        nc.gpsimd.affine_select(
            m64tmp, ones1k, pattern=[[1, 1024]], compare_op=ALU.is_ge,
            fill=0.0, base=0, channel_multiplier=-16,
        )
        mask64_sb = singles.tile([64, 1024], bf16, name="mask64_sb")
        nc.gpsimd.affine_select(
            mask64_sb, m64tmp, pattern=[[-1, 1024]], compare_op=ALU.is_ge,
            fill=0.0, base=15, channel_multiplier=16,
        )
        ones64b_sb = singles.tile([64, 1], bf16, name="ones64b_sb")
        nc.vector.memset(ones64b_sb, 1.0)

        # -------- feature loads (jb=0 chunk of f2 first) --------
        f1T = singles.tile([128, 2, RPC], f8, name="f1T")
        f2a = singles.tile([128, 2, GN], f8, name="f2a")
        for kc in (0, 1):
            nc.sync.dma_start(
                out=f1T[:, kc, :],
                in_=fin_d[kc * 128 : (kc + 1) * 128, 0:RPC],
            )
        for kc in (0, 1):
            nc.sync.dma_start(
                out=f2a[:, kc, 0:1024],
                in_=fin_d[kc * 128 : (kc + 1) * 128, RPC : RPC + 1024],
            )
            nc.gpsimd.dma_start(
                out=f2a[:, kc, 1024:GN],
                in_=fin_d[kc * 128 : (kc + 1) * 128, RPC + 1024 : RPC + GN],
            )

        rowblk = [
            singles.tile([128, G], bf16, name=f"rowblk{t}", tag=f"rowblk{t}")
            for t in range(NSTRIP)
        ]
        o_rows_sb = singles.tile([128, 3 * NSTRIP], f32, name="o_rows_sb")
        o_pos2_sb = singles.tile([1, RPC], f32, name="o_pos2_sb")
        o_ca_sb = singles.tile([1, GN], f32, name="o_ca_sb")
        o_cb_sb = singles.tile([1, GN], f32, name="o_cb_sb")
        posv = singles.tile([128, NSTRIP], f32, name="posv")
        craw_all = [
            singles.tile([64, 1024], bf16, name=f"craw{jb}", tag=f"craw{jb}")
            for jb in range(NJB)
        ]

        with tc.tile_pool(name="psg", bufs=2, space="PSUM") as psg, \
             tc.tile_pool(name="psc", bufs=2, space="PSUM") as psc:
            # -------- main fused GEMM + stats loop -----------------------
            for jb in range(NJB):
                colps = psc.tile([64, 1024], f32, tag="colps", name="colps")
                for t in range(NSTRIP):
                    ps = psg.tile([128, 1024], f32, tag="gemm", name="ps")
                    for h in (0, 1):
                        nc.tensor.matmul(
                            ps[:, h * 512 : (h + 1) * 512],
                            lhsT=f1T[:, :, t * 128 : (t + 1) * 128],
                            rhs=f2a[
                                :, :, jb * 1024 + h * 512 : jb * 1024 + (h + 1) * 512
                            ],
                            start=True,
                            stop=True,
                            perf_mode=mybir.MatmulPerfMode.DoubleRow,
                        )
                    expb = expp.tile([128, 1024], bf16, tag="exp", name="expb")
                    nc.scalar.activation(expb, ps, AF.Exp, scale=scale / (PRE * PRE))
                    with nc.allow_low_precision(
                        reason="16-wide bf16 blocksums; loss err ~1e-4"
                    ):
                        nc.vector.reduce_sum(
                            out=rowblk[t][:, jb * 64 : (jb + 1) * 64],
                            in_=expb.rearrange("p (g n) -> p g n", n=NGRP),
                            axis=AX.X,
                        )
                    for h in (0, 1):
                        nc.tensor.matmul(
                            colps[:, h * 512 : (h + 1) * 512],
                            lhsT=ones64_sb[:, t, :],
                            rhs=expb[:, h * 512 : (h + 1) * 512],
                            start=(t == 0),
                            stop=(t == NSTRIP - 1),
                        )
                with nc.allow_low_precision(
                    reason="bf16 column blocksums; loss err ~1e-4"
                ):
                    nc.vector.tensor_copy(craw_all[jb], colps)

            # -------- row positives from rowblk's jb=0 columns -----------
            for t in range(NSTRIP):
                pose = scratch.tile([128, 8], f32, tag="pose", name="pose")
                nc.vector.tensor_mul(
                    pose, rowblk[t][:, 8 * t : 8 * t + 8], mask128_sb
                )
                nc.vector.reduce_sum(
                    out=posv[:, t : t + 1], in_=pose, axis=AX.X
                )

            # -------- deferred log-domain tails (single Exp->Ln swap) ----
            for t in range(NSTRIP):
                nc.vector.reduce_sum(
                    out=o_rows_sb[:, t : t + 1], in_=rowblk[t], axis=AX.X
                )
            # late_zero is data-dependent on the last row reduction, which
            # pins the early-ready Ln ops below AFTER the main Exp stream so
            # the scheduler cannot interleave activation-table reloads
            late_zero = singles.tile([128, 1], f32, name="late_zero")
            nc.vector.tensor_scalar_mul(
                late_zero, o_rows_sb[:, NSTRIP - 1 : NSTRIP], 0.0
            )
            nc.scalar.activation(
                o_rows_sb[:, 2 * NSTRIP : 3 * NSTRIP], posv, AF.Ln,
                bias=late_zero,
            )
            for t in range(NSTRIP):
                with nc.allow_low_precision(
                    reason="Ln output tile unused; accum_out is f32"
                ):
                    nc.scalar.activation(
                        rowblk[t], rowblk[t], AF.Ln, bias=late_zero,
                        accum_out=o_rows_sb[:, NSTRIP + t : NSTRIP + t + 1],
                    )
            # column positives: craw0 masked to its block diagonal, then
            # 64->1 ones-matmul (exactly one nonzero per column)
            pcm = crawp.tile([64, 1024], bf16, tag="pcm", name="pcm")
            nc.vector.tensor_mul(pcm, craw_all[0], mask64_sb)
            pcps = psc.tile([64, 1024], f32, tag="colps", name="pcps")
            for h in (0, 1):
                nc.tensor.matmul(
                    pcps[0:1, h * 512 : (h + 1) * 512],
                    lhsT=ones64b_sb,
                    rhs=pcm[:, h * 512 : (h + 1) * 512],
                    start=True,
                    stop=True,
                )
            nc.scalar.activation(
                o_pos2_sb, pcps[0:1, :], AF.Ln, bias=late_zero[0:1, :]
            )
            for jb in range(NJB):
                craw_sb = craw_all[jb]
                blog_sb = crawp.tile([64, 1024], bf16, tag="pcm", name="blog_sb")
                with nc.allow_low_precision(
                    reason="bf16 log blocksums; weight eps/G is tiny"
                ):
                    nc.scalar.activation(
                        blog_sb, craw_sb, AF.Ln, bias=late_zero[0:64, :]
                    )
                # partition-reduce (64 groups -> 1) as bf16 ones-matmuls:
                # partition 0 = colsum(craw), partition 32 = colsum(blog)
                cbps = psc.tile([64, 1024], f32, tag="colps", name="cbps")
                for h in (0, 1):
                    nc.tensor.matmul(
                        cbps[0:1, h * 512 : (h + 1) * 512],
                        lhsT=ones64b_sb,
                        rhs=craw_sb[:, h * 512 : (h + 1) * 512],
                        start=True,
                        stop=True,
                    )
                    nc.tensor.matmul(
                        cbps[32:33, h * 512 : (h + 1) * 512],
                        lhsT=ones64b_sb,
                        rhs=blog_sb[:, h * 512 : (h + 1) * 512],
                        start=True,
                        stop=True,
                    )
                nc.vector.tensor_copy(
                    o_ca_sb[:, jb * 1024 : (jb + 1) * 1024], cbps[0:1, :]
                )
                nc.vector.tensor_copy(
                    o_cb_sb[:, jb * 1024 : (jb + 1) * 1024], cbps[32:33, :]
                )

        nc.sync.dma_start(out=o_rows_d, in_=o_rows_sb)
        nc.sync.dma_start(out=o_pos2_d, in_=o_pos2_sb)
        nc.sync.dma_start(out=o_cab_d[0:1, :], in_=o_ca_sb)
        nc.sync.dma_start(out=o_cab_d[1:2, :], in_=o_cb_sb)

    nc.compile()
    return nc


def build_in_maps(image_features1, image_features2, logit_scale):
    """Host prep: normalize, fp8-quantize, transpose, shard f1 / rotate f2."""
    import ml_dtypes

    f1 = np.asarray(image_features1, dtype=np.float32)
    f2 = np.asarray(image_features2, dtype=np.float32)
    s = float(np.asarray(logit_scale).reshape(-1)[0])

    f1n = f1 / np.linalg.norm(f1, axis=-1, keepdims=True)
    f2n = f2 / np.linalg.norm(f2, axis=-1, keepdims=True)
    PRE = 16.0
    f1nT = np.ascontiguousarray((f1n.T * PRE).astype(ml_dtypes.float8_e4m3))
    f2nT = np.ascontiguousarray((f2n.T * PRE).astype(ml_dtypes.float8_e4m3))

    in_maps = []
    for k in range(NCORES):
        fin = np.empty((D, RPC + GN), dtype=ml_dtypes.float8_e4m3)
        fin[:, :RPC] = f1nT[:, k * RPC : (k + 1) * RPC]
        # rotate so local col j maps to global col (j + RPC*k) % GN
        fin[:, RPC : RPC + GN - k * RPC] = f2nT[:, k * RPC :]
        fin[:, RPC + GN - k * RPC :] = f2nT[:, : k * RPC]
        in_maps.append({"fin": fin})
    return s, in_maps


def combine_host(results):
    """O(GN) host combine of per-core row/column stats -> scalar loss."""
    eps = EPS
    S1 = 0.0
    for k in range(NCORES):
        r = results[k]["o_rows"].astype(np.float64)
        asum = r[:, 0:NSTRIP]            # [128, 8] sum_j exp
        slog = r[:, NSTRIP : 2 * NSTRIP]  # [128, 8] sum_g log blocksum
        pos = r[:, 2 * NSTRIP : 3 * NSTRIP]  # [128, 8] log blocksum at pos
        per_row = np.log(asum) - (1.0 - eps) * pos - (eps / G) * slog
        S1 += per_row.sum()

    a_tot = np.zeros(GN, dtype=np.float64)
    b_tot = np.zeros(GN, dtype=np.float64)
    lpos2 = np.zeros(GN, dtype=np.float64)
    for k in range(NCORES):
        cab = results[k]["o_cab"].astype(np.float64)
        # local col j holds global col (j + RPC*k) % GN -> roll right by RPC*k
        a_tot += np.roll(cab[0], RPC * k)
        b_tot += np.roll(cab[1], RPC * k)
        lpos2[k * RPC : (k + 1) * RPC] = (
            results[k]["o_pos2"].astype(np.float64).ravel()
        )
    per_row2 = np.log(a_tot) - (1.0 - eps) * lpos2 - (eps / G) * b_tot
    S2 = per_row2.sum()

    return (S1 + S2) / (2.0 * GN)


def kernel(image_features1, image_features2, logit_scale):
    global last_results
    from concourse.bass_utils import run_bass_kernel_spmd

    s, in_maps = build_in_maps(image_features1, image_features2, logit_scale)

    key = round(s, 9)
    if key not in _cache:
        _cache[key] = _build_program(s)
    nc = _cache[key]

    try:
        res = run_bass_kernel_spmd(
            nc,
            in_maps,
            core_ids=list(range(NCORES)),
            trace=bool(os.environ.get("KTRACE")),
        )
    except ModuleNotFoundError:
        # axon build without NTFF profiling hooks — rerun without trace
        res = run_bass_kernel_spmd(
            nc, in_maps, core_ids=list(range(NCORES)), trace=False
        )
    last_results = res

    loss = combine_host(res.results)
    return np.array(loss, dtype=np.float32)


# revision 10
# speedup vs baseline: 1.5332x; 1.5332x over previous
"""GroupInfoNCE loss kernel for 8 Trainium2 NeuronCores.

Strategy (row-sharded, f2 replicated + column-rotated, no collectives):
  - Host normalizes f1/f2 in f32, pre-scales x16, quantizes to fp8e4m3,
    transposes to [D, rows]. Core k receives its f1 row-shard [256, 1024]
    plus the FULL f2T [256, 8192] with columns rotated left by 1024*k, so
    every core runs an identical program whose jb=0 column block is its
    own diagonal (positive) block. No AllGather, no collective rendezvous.
  - The 8192x8192 logits matrix never touches HBM: each [128,1024] GEMM
    tile (fp8 DoubleRow) is consumed in PSUM by ScalarE exp -> bf16.
  - Row stats: VectorE 3D-AP reduce -> 16-wide block sums rowblk; row
    positives come from rowblk's jb=0 columns via a mask-multiply.
  - Column stats: TensorE ones-matmul -> per-core column block sums craw
    [64, 1024] per jb (bf16, copied PSUM->SBUF on the idle Pool engine);
    64->1 partition reduction via tiny bf16 ones-matmuls on PE. Column
    positives = craw0 masked to its block-diagonal, same ones-matmul.
  - All Ln ops are pinned after the Exp stream via a late-data bias
    operand so ScalarE swaps activation tables exactly once.
  - Host does the tiny O(GN) combine (un-rotating column stats).
"""

import os
import numpy as np

GN, D = 8192, 256
NGRP = 16               # group length N
EPS = 0.1               # label smoothing
G = GN // NGRP          # 512 groups
NCORES = 8
RPC = GN // NCORES      # 1024 rows per core
NSTRIP = RPC // 128     # 8 strips of 128 rows
NJB = GN // 1024        # 8 j-blocks of 1024 columns

_cache = {}
last_results = None


def _build_program(scale: float):
    PRE = 16.0  # host fp8 pre-scale; folded out of the exp activation scale
    from contextlib import ExitStack
    import concourse.bass as bass  # noqa: F401
    import concourse.mybir as mybir
    import concourse.tile as tile
    from concourse import bacc

    f32 = mybir.dt.float32
    bf16 = mybir.dt.bfloat16
    f8 = mybir.dt.float8e4
    AF = mybir.ActivationFunctionType
    AX = mybir.AxisListType
    ALU = mybir.AluOpType

    nc = bacc.Bacc(
        "TRN2",
        target_bir_lowering=False,
        debug=False,
        enable_asserts=False,
        num_devices=NCORES,
    )

    # packed input: cols [0, RPC) = f1T shard, cols [RPC, RPC+GN) = rotated f2T
    fin_d = nc.dram_tensor("fin", [D, RPC + GN], f8, kind="ExternalInput").ap()

    # packed outputs: o_rows = [asum | slog | pos] column-wise
    o_rows_d = nc.dram_tensor(
        "o_rows", [128, 3 * NSTRIP], f32, kind="ExternalOutput"
    ).ap()
    o_pos2_d = nc.dram_tensor("o_pos2", [1, RPC], f32, kind="ExternalOutput").ap()
    o_cab_d = nc.dram_tensor("o_cab", [2, GN], f32, kind="ExternalOutput").ap()

    with tile.TileContext(nc) as tc, ExitStack() as ctx:
        singles = ctx.enter_context(tc.tile_pool(name="singles", bufs=1))
        expp = ctx.enter_context(tc.tile_pool(name="expp", bufs=8))
        crawp = ctx.enter_context(tc.tile_pool(name="crawp", bufs=2))
        scratch = ctx.enter_context(tc.tile_pool(name="scratch", bufs=2))

        # -------- constants built on device (no input bytes) --------
        # mask128[p, g] = 1 iff g == p//16, via two affine selects on the
        # iota value v(p, g) = p - 16g (keep 0 <= v <= 15)
        ones8 = singles.tile([128, 8], f32, name="ones8")
        nc.vector.memset(ones8, 1.0)
        mtmp = singles.tile([128, 8], f32, name="mtmp")
        nc.gpsimd.affine_select(
            mtmp, ones8, pattern=[[-16, 8]], compare_op=ALU.is_ge,
            fill=0.0, base=0, channel_multiplier=1,
        )
        mask128_sb = singles.tile([128, 8], f32, name="mask128_sb")
        nc.gpsimd.affine_select(
            mask128_sb, mtmp, pattern=[[16, 8]], compare_op=ALU.is_ge,
            fill=0.0, base=15, channel_multiplier=-1,
        )
        # ones64[p, t, c] = 1 iff c == 8t + p//16  (colsum weights per strip)
        ones64_sb = singles.tile([128, NSTRIP, 64], bf16, name="ones64_sb")
        nc.vector.memset(ones64_sb, 0.0)
        for t in range(NSTRIP):
            nc.vector.tensor_copy(
                ones64_sb[:, t, 8 * t : 8 * t + 8], mask128_sb
            )
        # mask64[g, c] = 1 iff c//16 == g (extracts craw0's positive entries)
        ones1k = singles.tile([64, 1024], bf16, name="ones1k")
        nc.vector.memset(ones1k, 1.0)
        m64tmp = singles.tile([64, 1024], bf16, name="m64tmp")
        nc.gpsimd.affine_select(
            m64tmp, ones1k, pattern=[[1, 1024]], compare_op=ALU.is_ge,
            fill=0.0, base=0, channel_multiplier=-16,
        )
        mask64_sb = singles.tile([64, 1024], bf16, name="mask64_sb")
        nc.gpsimd.affine_select(
            mask64_sb, m64tmp, pattern=[[-1, 1024]], compare_op=ALU.is_ge,
            fill=0.0, base=15, channel_multiplier=16,
        )
        ones64b_sb = singles.tile([64, 1], bf16, name="ones64b_sb")
        nc.vector.memset(ones64b_sb, 1.0)

        # -------- feature loads (jb=0 chunk of f2 first) --------
        f1T = singles.tile([128, 2, RPC], f8, name="f1T")
        f2a = singles.tile([128, 2, GN], f8, name="f2a")
        for kc in (0, 1):
            nc.sync.dma_start(
                out=f1T[:, kc, :],
                in_=fin_d[kc * 128 : (kc + 1) * 128, 0:RPC],
            )
        for kc in (0, 1):
            nc.sync.dma_start(
                out=f2a[:, kc, 0:1024],
                in_=fin_d[kc * 128 : (kc + 1) * 128, RPC : RPC + 1024],
            )
            nc.gpsimd.dma_start(
                out=f2a[:, kc, 1024:GN],
                in_=fin_d[kc * 128 : (kc + 1) * 128, RPC + 1024 : RPC + GN],
            )

        rowblk = [
            singles.tile([128, G], bf16, name=f"rowblk{t}", tag=f"rowblk{t}")
            for t in range(NSTRIP)
        ]
        o_rows_sb = singles.tile([128, 3 * NSTRIP], f32, name="o_rows_sb")
        o_pos2_sb = singles.tile([1, RPC], f32, name="o_pos2_sb")
        posv = singles.tile([128, NSTRIP], f32, name="posv")
        craw_all = [
            singles.tile([64, 1024], bf16, name=f"craw{jb}", tag=f"craw{jb}")
            for jb in range(NJB)
        ]

        with tc.tile_pool(name="psg", bufs=2, space="PSUM") as psg, \
             tc.tile_pool(name="psc", bufs=2, space="PSUM") as psc:
            # -------- main fused GEMM + stats loop -----------------------
            for jb in range(NJB):
                colps = psc.tile([64, 1024], f32, tag="colps", name="colps")
                for t in range(NSTRIP):
                    ps = psg.tile([128, 1024], f32, tag="gemm", name="ps")
                    for h in (0, 1):
                        nc.tensor.matmul(
                            ps[:, h * 512 : (h + 1) * 512],
                            lhsT=f1T[:, :, t * 128 : (t + 1) * 128],
                            rhs=f2a[
                                :, :, jb * 1024 + h * 512 : jb * 1024 + (h + 1) * 512
                            ],
                            start=True,
                            stop=True,
                            perf_mode=mybir.MatmulPerfMode.DoubleRow,
                        )
                    expb = expp.tile([128, 1024], bf16, tag="exp", name="expb")
                    nc.scalar.activation(expb, ps, AF.Exp, scale=scale / (PRE * PRE))
                    with nc.allow_low_precision(
                        reason="16-wide bf16 blocksums; loss err ~1e-4"
                    ):
                        nc.vector.reduce_sum(
                            out=rowblk[t][:, jb * 64 : (jb + 1) * 64],
                            in_=expb.rearrange("p (g n) -> p g n", n=NGRP),
                            axis=AX.X,
                        )
                    for h in (0, 1):
                        nc.tensor.matmul(
                            colps[:, h * 512 : (h + 1) * 512],
                            lhsT=ones64_sb[:, t, :],
                            rhs=expb[:, h * 512 : (h + 1) * 512],
                            start=(t == 0),
                            stop=(t == NSTRIP - 1),
                        )
                with nc.allow_low_precision(
                    reason="bf16 column blocksums; loss err ~1e-4"
                ):
                    nc.vector.tensor_copy(craw_all[jb], colps)

            # -------- row positives from rowblk's jb=0 columns -----------
            for t in range(NSTRIP):
                pose = scratch.tile([128, 8], f32, tag="pose", name="pose")
                nc.vector.tensor_mul(
                    pose, rowblk[t][:, 8 * t : 8 * t + 8], mask128_sb
                )
                nc.vector.reduce_sum(
                    out=posv[:, t : t + 1], in_=pose, axis=AX.X
                )

            # -------- deferred log-domain tails (single Exp->Ln swap) ----
            for t in range(NSTRIP):
                nc.vector.reduce_sum(
                    out=o_rows_sb[:, t : t + 1], in_=rowblk[t], axis=AX.X
                )
            # late_zero is data-dependent on the last row reduction, which
            # pins the early-ready Ln ops below AFTER the main Exp stream so
            # the scheduler cannot interleave activation-table reloads
            late_zero = singles.tile([128, 1], f32, name="late_zero")
            nc.vector.tensor_scalar_mul(
                late_zero, o_rows_sb[:, NSTRIP - 1 : NSTRIP], 0.0
            )
            nc.scalar.activation(
                o_rows_sb[:, 2 * NSTRIP : 3 * NSTRIP], posv, AF.Ln,
                bias=late_zero,
            )
            for t in range(NSTRIP):
                with nc.allow_low_precision(
                    reason="Ln output tile unused; accum_out is f32"
                ):
                    nc.scalar.activation(
                        rowblk[t], rowblk[t], AF.Ln, bias=late_zero,
                        accum_out=o_rows_sb[:, NSTRIP + t : NSTRIP + t + 1],
                    )
            # column positives: craw0 masked to its block diagonal, then
            # 64->1 ones-matmul (exactly one nonzero per column)
            pcm = crawp.tile([64, 1024], bf16, tag="pcm", name="pcm")
            nc.vector.tensor_mul(pcm, craw_all[0], mask64_sb)
            pcps = psc.tile([64, 1024], f32, tag="colps", name="pcps")
            for h in (0, 1):
                nc.tensor.matmul(
                    pcps[0:1, h * 512 : (h + 1) * 512],
                    lhsT=ones64b_sb,
                    rhs=pcm[:, h * 512 : (h + 1) * 512],
                    start=True,
                    stop=True,
                )
            nc.scalar.activation(
                o_pos2_sb, pcps[0:1, :], AF.Ln, bias=late_zero[0:1, :]
            )
            for jb in range(NJB):
                craw_sb = craw_all[jb]
                blog_sb = crawp.tile([64, 1024], bf16, tag="pcm", name="blog_sb")
                with nc.allow_low_precision(
                    reason="bf16 log blocksums; weight eps/G is tiny"
                ):
                    nc.scalar.activation(
                        blog_sb, craw_sb, AF.Ln, bias=late_zero[0:64, :]
                    )
                # partition-reduce (64 groups -> 1) as bf16 ones-matmuls:
                # partition 0 = colsum(craw), partition 32 = colsum(blog)
                cbps = psc.tile([64, 1024], f32, tag="colps", name="cbps")
                for h in (0, 1):
                    nc.tensor.matmul(
                        cbps[0:1, h * 512 : (h + 1) * 512],
                        lhsT=ones64b_sb,
                        rhs=craw_sb[:, h * 512 : (h + 1) * 512],
                        start=True,
                        stop=True,
                    )
                    nc.tensor.matmul(
                        cbps[32:33, h * 512 : (h + 1) * 512],
                        lhsT=ones64b_sb,
                        rhs=blog_sb[:, h * 512 : (h + 1) * 512],
                        start=True,
                        stop=True,
                    )
                # one copy spanning partitions 0..32 costs the same DVE
                # cycles as a single row (cost = free size); DMA rows 0/32
                cbst = crawp.tile([33, 1024], f32, tag="cbst", name="cbst")
                nc.vector.tensor_copy(cbst, cbps[0:33, :])
                nc.sync.dma_start(
                    out=o_cab_d[0:1, jb * 1024 : (jb + 1) * 1024],
                    in_=cbst[0:1, :],
                )
                nc.gpsimd.dma_start(
                    out=o_cab_d[1:2, jb * 1024 : (jb + 1) * 1024],
                    in_=cbst[32:33, :],
                )

        nc.sync.dma_start(out=o_rows_d, in_=o_rows_sb)
        nc.sync.dma_start(out=o_pos2_d, in_=o_pos2_sb)

    nc.compile()
    return nc


def build_in_maps(image_features1, image_features2, logit_scale):
    """Host prep: normalize, fp8-quantize, transpose, shard f1 / rotate f2."""
    import ml_dtypes

    f1 = np.asarray(image_features1, dtype=np.float32)
    f2 = np.asarray(image_features2, dtype=np.float32)
    s = float(np.asarray(logit_scale).reshape(-1)[0])

    f1n = f1 / np.linalg.norm(f1, axis=-1, keepdims=True)
    f2n = f2 / np.linalg.norm(f2, axis=-1, keepdims=True)
    PRE = 16.0
    f1nT = np.ascontiguousarray((f1n.T * PRE).astype(ml_dtypes.float8_e4m3))
    f2nT = np.ascontiguousarray((f2n.T * PRE).astype(ml_dtypes.float8_e4m3))

    in_maps = []
    for k in range(NCORES):
        fin = np.empty((D, RPC + GN), dtype=ml_dtypes.float8_e4m3)
        fin[:, :RPC] = f1nT[:, k * RPC : (k + 1) * RPC]
        # rotate so local col j maps to global col (j + RPC*k) % GN
        fin[:, RPC : RPC + GN - k * RPC] = f2nT[:, k * RPC :]
        fin[:, RPC + GN - k * RPC :] = f2nT[:, : k * RPC]
        in_maps.append({"fin": fin})
    return s, in_maps


def combine_host(results):
    """O(GN) host combine of per-core row/column stats -> scalar loss."""
    eps = EPS
    S1 = 0.0
    for k in range(NCORES):
        r = results[k]["o_rows"].astype(np.float64)
        asum = r[:, 0:NSTRIP]            # [128, 8] sum_j exp
        slog = r[:, NSTRIP : 2 * NSTRIP]  # [128, 8] sum_g log blocksum
        pos = r[:, 2 * NSTRIP : 3 * NSTRIP]  # [128, 8] log blocksum at pos
        per_row = np.log(asum) - (1.0 - eps) * pos - (eps / G) * slog
        S1 += per_row.sum()

    a_tot = np.zeros(GN, dtype=np.float64)
    b_tot = np.zeros(GN, dtype=np.float64)
    lpos2 = np.zeros(GN, dtype=np.float64)
    for k in range(NCORES):
        cab = results[k]["o_cab"].astype(np.float64)
        # local col j holds global col (j + RPC*k) % GN -> roll right by RPC*k
        a_tot += np.roll(cab[0], RPC * k)
        b_tot += np.roll(cab[1], RPC * k)
        lpos2[k * RPC : (k + 1) * RPC] = (
            results[k]["o_pos2"].astype(np.float64).ravel()
        )
    per_row2 = np.log(a_tot) - (1.0 - eps) * lpos2 - (eps / G) * b_tot
    S2 = per_row2.sum()

    return (S1 + S2) / (2.0 * GN)


def kernel(image_features1, image_features2, logit_scale):
    global last_results
    from concourse.bass_utils import run_bass_kernel_spmd

    s, in_maps = build_in_maps(image_features1, image_features2, logit_scale)

    key = round(s, 9)
    if key not in _cache:
        _cache[key] = _build_program(s)
    nc = _cache[key]

    try:
        res = run_bass_kernel_spmd(
            nc,
            in_maps,
            core_ids=list(range(NCORES)),
            trace=bool(os.environ.get("KTRACE")),
        )
    except ModuleNotFoundError:
        # axon build without NTFF profiling hooks — rerun without trace
        res = run_bass_kernel_spmd(
            nc, in_maps, core_ids=list(range(NCORES)), trace=False
        )
    last_results = res

    loss = combine_host(res.results)
    return np.array(loss, dtype=np.float32)


# revision 11
# speedup vs baseline: 2.3129x; 1.5086x over previous
"""GroupInfoNCE loss kernel for 8 Trainium2 NeuronCores.

Strategy (row-sharded, f2 replicated + column-rotated, no collectives):
  - Host normalizes f1/f2 in f32, pre-scales x16, quantizes to fp8e4m3,
    transposes to [D, rows]. Core k receives its f1 row-shard [256, 1024]
    plus the FULL f2T [256, 8192] with columns rotated left by 1024*k, so
    every core runs an identical program whose jb=0 column block is its
    own diagonal (positive) block. No AllGather, no collective rendezvous.
  - The 8192x8192 logits matrix never touches HBM: each [128,1024] GEMM
    tile (fp8 DoubleRow) is consumed in PSUM by ScalarE exp -> bf16.
  - Row stats: VectorE 3D-AP reduce -> 16-wide block sums rowblk; row
    positives come from rowblk's jb=0 columns via a mask-multiply.
  - Column stats: TensorE ones-matmul -> per-core column block sums craw
    [64, 1024] per jb (bf16, copied PSUM->SBUF on the idle Pool engine);
    64->1 partition reduction via tiny bf16 ones-matmuls on PE. Column
    positives = craw0 masked to its block-diagonal, same ones-matmul.
  - All Ln ops are pinned after the Exp stream via a late-data bias
    operand so ScalarE swaps activation tables exactly once.
  - Host does the tiny O(GN) combine (un-rotating column stats).
"""

import os
import numpy as np

GN, D = 8192, 256
NGRP = 16               # group length N
EPS = 0.1               # label smoothing
G = GN // NGRP          # 512 groups
NCORES = 8
RPC = GN // NCORES      # 1024 rows per core
NSTRIP = RPC // 128     # 8 strips of 128 rows
NJB = GN // 1024        # 8 j-blocks of 1024 columns

_cache = {}
last_results = None


def _build_program(scale: float):
    PRE = 16.0  # host fp8 pre-scale; folded out of the exp activation scale
    from contextlib import ExitStack
    import concourse.bass as bass  # noqa: F401
    import concourse.mybir as mybir
    import concourse.tile as tile
    from concourse import bacc

    f32 = mybir.dt.float32
    bf16 = mybir.dt.bfloat16
    f8 = mybir.dt.float8e4
    AF = mybir.ActivationFunctionType
    AX = mybir.AxisListType
    ALU = mybir.AluOpType

    nc = bacc.Bacc(
        "TRN2",
        target_bir_lowering=False,
        debug=False,
        enable_asserts=False,
        num_devices=NCORES,
    )

    # packed input: cols [0, RPC) = f1T shard, cols [RPC, RPC+GN) = rotated f2T
    fin_d = nc.dram_tensor("fin", [D, RPC + GN], f8, kind="ExternalInput").ap()

    # single packed output (fewer result buffers = cheaper dispatch):
    # [0:3072] o_rows [128, 24] row-major; [3072:4096] o_pos2 [1, 1024];
    # [4096:12288] o_ca; [12288:20480] o_cb
    o_all_d = nc.dram_tensor(
        "o_all", [1, 3072 + RPC + 2 * GN], f32, kind="ExternalOutput"
    ).ap()

    with tile.TileContext(nc) as tc, ExitStack() as ctx:
        singles = ctx.enter_context(tc.tile_pool(name="singles", bufs=1))
        expp = ctx.enter_context(tc.tile_pool(name="expp", bufs=8))
        crawp = ctx.enter_context(tc.tile_pool(name="crawp", bufs=2))
        scratch = ctx.enter_context(tc.tile_pool(name="scratch", bufs=2))

        # -------- constants built on device (no input bytes) --------
        # mask128[p, g] = 1 iff g == p//16, via two affine selects on the
        # iota value v(p, g) = p - 16g (keep 0 <= v <= 15)
        ones8 = singles.tile([128, 8], f32, name="ones8")
        nc.vector.memset(ones8, 1.0)
        mtmp = singles.tile([128, 8], f32, name="mtmp")
        nc.gpsimd.affine_select(
            mtmp, ones8, pattern=[[-16, 8]], compare_op=ALU.is_ge,
            fill=0.0, base=0, channel_multiplier=1,
        )
        mask128_sb = singles.tile([128, 8], f32, name="mask128_sb")
        nc.gpsimd.affine_select(
            mask128_sb, mtmp, pattern=[[16, 8]], compare_op=ALU.is_ge,
            fill=0.0, base=15, channel_multiplier=-1,
        )
        # ones64[p, t, c] = 1 iff c == 8t + p//16  (colsum weights per strip)
        ones64_sb = singles.tile([128, NSTRIP, 64], bf16, name="ones64_sb")
        nc.vector.memset(ones64_sb, 0.0)
        for t in range(NSTRIP):
            nc.vector.tensor_copy(
                ones64_sb[:, t, 8 * t : 8 * t + 8], mask128_sb
            )
        # mask64[g, c] = 1 iff c//16 == g (extracts craw0's positive entries)
        ones1k = singles.tile([64, 1024], bf16, name="ones1k")
        nc.vector.memset(ones1k, 1.0)
        m64tmp = singles.tile([64, 1024], bf16, name="m64tmp")
        nc.gpsimd.affine_select(
            m64tmp, ones1k, pattern=[[1, 1024]], compare_op=ALU.is_ge,
            fill=0.0, base=0, channel_multiplier=-16,
        )
        mask64_sb = singles.tile([64, 1024], bf16, name="mask64_sb")
        nc.gpsimd.affine_select(
            mask64_sb, m64tmp, pattern=[[-1, 1024]], compare_op=ALU.is_ge,
            fill=0.0, base=15, channel_multiplier=16,
        )
        ones64b_sb = singles.tile([64, 1], bf16, name="ones64b_sb")
        nc.vector.memset(ones64b_sb, 1.0)

        # -------- feature loads (jb=0 chunk of f2 first) --------
        f1T = singles.tile([128, 2, RPC], f8, name="f1T")
        f2a = singles.tile([128, 2, GN], f8, name="f2a")
        for kc in (0, 1):
            nc.sync.dma_start(
                out=f1T[:, kc, :],
                in_=fin_d[kc * 128 : (kc + 1) * 128, 0:RPC],
            )
        for kc in (0, 1):
            nc.sync.dma_start(
                out=f2a[:, kc, 0:1024],
                in_=fin_d[kc * 128 : (kc + 1) * 128, RPC : RPC + 1024],
            )
            nc.gpsimd.dma_start(
                out=f2a[:, kc, 1024:GN],
                in_=fin_d[kc * 128 : (kc + 1) * 128, RPC + 1024 : RPC + GN],
            )

        rowblk = [
            singles.tile([128, G], bf16, name=f"rowblk{t}", tag=f"rowblk{t}")
            for t in range(NSTRIP)
        ]
        o_rows_sb = singles.tile([128, 3 * NSTRIP], f32, name="o_rows_sb")
        o_pos2_sb = singles.tile([1, RPC], f32, name="o_pos2_sb")
        posv = singles.tile([128, NSTRIP], f32, name="posv")
        craw_all = [
            singles.tile([64, 1024], bf16, name=f"craw{jb}", tag=f"craw{jb}")
            for jb in range(NJB)
        ]

        with tc.tile_pool(name="psg", bufs=2, space="PSUM") as psg, \
             tc.tile_pool(name="psc", bufs=2, space="PSUM") as psc:
            # -------- main fused GEMM + stats loop -----------------------
            for jb in range(NJB):
                colps = psc.tile([64, 1024], f32, tag="colps", name="colps")
                for t in range(NSTRIP):
                    ps = psg.tile([128, 1024], f32, tag="gemm", name="ps")
                    for h in (0, 1):
                        nc.tensor.matmul(
                            ps[:, h * 512 : (h + 1) * 512],
                            lhsT=f1T[:, :, t * 128 : (t + 1) * 128],
                            rhs=f2a[
                                :, :, jb * 1024 + h * 512 : jb * 1024 + (h + 1) * 512
                            ],
                            start=True,
                            stop=True,
                            perf_mode=mybir.MatmulPerfMode.DoubleRow,
                        )
                    expb = expp.tile([128, 1024], bf16, tag="exp", name="expb")
                    nc.scalar.activation(expb, ps, AF.Exp, scale=scale / (PRE * PRE))
                    with nc.allow_low_precision(
                        reason="16-wide bf16 blocksums; loss err ~1e-4"
                    ):
                        nc.vector.reduce_sum(
                            out=rowblk[t][:, jb * 64 : (jb + 1) * 64],
                            in_=expb.rearrange("p (g n) -> p g n", n=NGRP),
                            axis=AX.X,
                        )
                    for h in (0, 1):
                        nc.tensor.matmul(
                            colps[:, h * 512 : (h + 1) * 512],
                            lhsT=ones64_sb[:, t, :],
                            rhs=expb[:, h * 512 : (h + 1) * 512],
                            start=(t == 0),
                            stop=(t == NSTRIP - 1),
                        )
                with nc.allow_low_precision(
                    reason="bf16 column blocksums; loss err ~1e-4"
                ):
                    nc.vector.tensor_copy(craw_all[jb], colps)

            # -------- row positives from rowblk's jb=0 columns -----------
            for t in range(NSTRIP):
                pose = scratch.tile([128, 8], f32, tag="pose", name="pose")
                nc.vector.tensor_mul(
                    pose, rowblk[t][:, 8 * t : 8 * t + 8], mask128_sb
                )
                nc.vector.reduce_sum(
                    out=posv[:, t : t + 1], in_=pose, axis=AX.X
                )

            # -------- deferred log-domain tails (single Exp->Ln swap) ----
            for t in range(NSTRIP):
                nc.vector.reduce_sum(
                    out=o_rows_sb[:, t : t + 1], in_=rowblk[t], axis=AX.X
                )
            # late_zero is data-dependent on the last row reduction, which
            # pins the early-ready Ln ops below AFTER the main Exp stream so
            # the scheduler cannot interleave activation-table reloads
            late_zero = singles.tile([128, 1], f32, name="late_zero")
            nc.vector.tensor_scalar_mul(
                late_zero, o_rows_sb[:, NSTRIP - 1 : NSTRIP], 0.0
            )
            nc.scalar.activation(
                o_rows_sb[:, 2 * NSTRIP : 3 * NSTRIP], posv, AF.Ln,
                bias=late_zero,
            )
            for t in range(NSTRIP):
                with nc.allow_low_precision(
                    reason="Ln output tile unused; accum_out is f32"
                ):
                    nc.scalar.activation(
                        rowblk[t], rowblk[t], AF.Ln, bias=late_zero,
                        accum_out=o_rows_sb[:, NSTRIP + t : NSTRIP + t + 1],
                    )
            # column positives: craw0 masked to its block diagonal, then
            # 64->1 ones-matmul (exactly one nonzero per column)
            pcm = crawp.tile([64, 1024], bf16, tag="pcm", name="pcm")
            nc.vector.tensor_mul(pcm, craw_all[0], mask64_sb)
            pcps = psc.tile([64, 1024], f32, tag="colps", name="pcps")
            for h in (0, 1):
                nc.tensor.matmul(
                    pcps[0:1, h * 512 : (h + 1) * 512],
                    lhsT=ones64b_sb,
                    rhs=pcm[:, h * 512 : (h + 1) * 512],
                    start=True,
                    stop=True,
                )
            nc.scalar.activation(
                o_pos2_sb, pcps[0:1, :], AF.Ln, bias=late_zero[0:1, :]
            )
            for jb in range(NJB):
                craw_sb = craw_all[jb]
                blog_sb = crawp.tile([64, 1024], bf16, tag="pcm", name="blog_sb")
                with nc.allow_low_precision(
                    reason="bf16 log blocksums; weight eps/G is tiny"
                ):
                    nc.scalar.activation(
                        blog_sb, craw_sb, AF.Ln, bias=late_zero[0:64, :]
                    )
                # partition-reduce (64 groups -> 1) as bf16 ones-matmuls:
                # partition 0 = colsum(craw), partition 32 = colsum(blog)
                cbps = psc.tile([64, 1024], f32, tag="colps", name="cbps")
                for h in (0, 1):
                    nc.tensor.matmul(
                        cbps[0:1, h * 512 : (h + 1) * 512],
                        lhsT=ones64b_sb,
                        rhs=craw_sb[:, h * 512 : (h + 1) * 512],
                        start=True,
                        stop=True,
                    )
                    nc.tensor.matmul(
                        cbps[32:33, h * 512 : (h + 1) * 512],
                        lhsT=ones64b_sb,
                        rhs=blog_sb[:, h * 512 : (h + 1) * 512],
                        start=True,
                        stop=True,
                    )
                # one copy spanning partitions 0..32 costs the same DVE
                # cycles as a single row (cost = free size); DMA rows 0/32
                cbst = crawp.tile([33, 1024], f32, tag="cbst", name="cbst")
                nc.vector.tensor_copy(cbst, cbps[0:33, :])
                nc.sync.dma_start(
                    out=o_all_d[0:1, 4096 + jb * 1024 : 4096 + (jb + 1) * 1024],
                    in_=cbst[0:1, :],
                )
                nc.gpsimd.dma_start(
                    out=o_all_d[
                        0:1, 12288 + jb * 1024 : 12288 + (jb + 1) * 1024
                    ],
                    in_=cbst[32:33, :],
                )

        nc.sync.dma_start(out=o_all_d[0:1, 0:3072], in_=o_rows_sb)
        nc.sync.dma_start(out=o_all_d[0:1, 3072:4096], in_=o_pos2_sb)

    nc.compile()
    return nc


def build_in_maps(image_features1, image_features2, logit_scale):
    """Host prep: normalize, fp8-quantize, transpose, shard f1 / rotate f2."""
    import ml_dtypes

    f1 = np.asarray(image_features1, dtype=np.float32)
    f2 = np.asarray(image_features2, dtype=np.float32)
    s = float(np.asarray(logit_scale).reshape(-1)[0])

    f1n = f1 / np.linalg.norm(f1, axis=-1, keepdims=True)
    f2n = f2 / np.linalg.norm(f2, axis=-1, keepdims=True)
    PRE = 16.0
    f1nT = np.ascontiguousarray((f1n.T * PRE).astype(ml_dtypes.float8_e4m3))
    f2nT = np.ascontiguousarray((f2n.T * PRE).astype(ml_dtypes.float8_e4m3))

    in_maps = []
    for k in range(NCORES):
        fin = np.empty((D, RPC + GN), dtype=ml_dtypes.float8_e4m3)
        fin[:, :RPC] = f1nT[:, k * RPC : (k + 1) * RPC]
        # rotate so local col j maps to global col (j + RPC*k) % GN
        fin[:, RPC : RPC + GN - k * RPC] = f2nT[:, k * RPC :]
        fin[:, RPC + GN - k * RPC :] = f2nT[:, : k * RPC]
        in_maps.append({"fin": fin})
    return s, in_maps


def combine_host(results):
    """O(GN) host combine of per-core row/column stats -> scalar loss."""
    eps = EPS
    S1 = 0.0
    a_tot = np.zeros(GN, dtype=np.float64)
    b_tot = np.zeros(GN, dtype=np.float64)
    lpos2 = np.zeros(GN, dtype=np.float64)
    for k in range(NCORES):
        flat = results[k]["o_all"].astype(np.float64).ravel()
        r = flat[0:3072].reshape(128, 3 * NSTRIP)
        asum = r[:, 0:NSTRIP]            # [128, 8] sum_j exp
        slog = r[:, NSTRIP : 2 * NSTRIP]  # [128, 8] sum_g log blocksum
        pos = r[:, 2 * NSTRIP : 3 * NSTRIP]  # [128, 8] log blocksum at pos
        per_row = np.log(asum) - (1.0 - eps) * pos - (eps / G) * slog
        S1 += per_row.sum()
        # local col j holds global col (j + RPC*k) % GN -> roll right by RPC*k
        a_tot += np.roll(flat[4096:12288], RPC * k)
        b_tot += np.roll(flat[12288:20480], RPC * k)
        lpos2[k * RPC : (k + 1) * RPC] = flat[3072:4096]
    per_row2 = np.log(a_tot) - (1.0 - eps) * lpos2 - (eps / G) * b_tot
    S2 = per_row2.sum()

    return (S1 + S2) / (2.0 * GN)


def kernel(image_features1, image_features2, logit_scale):
    global last_results
    from concourse.bass_utils import run_bass_kernel_spmd

    s, in_maps = build_in_maps(image_features1, image_features2, logit_scale)

    key = round(s, 9)
    if key not in _cache:
        _cache[key] = _build_program(s)
    nc = _cache[key]

    try:
        res = run_bass_kernel_spmd(
            nc,
            in_maps,
            core_ids=list(range(NCORES)),
            trace=bool(os.environ.get("KTRACE")),
        )
    except ModuleNotFoundError:
        # axon build without NTFF profiling hooks — rerun without trace
        res = run_bass_kernel_spmd(
            nc, in_maps, core_ids=list(range(NCORES)), trace=False
        )
    last_results = res

    loss = combine_host(res.results)
    return np.array(loss, dtype=np.float32)
